# revision 1
# baseline (speedup 1.0000x reference)
"""Low-rank attention kernel for Trainium2, distributed over 8 NeuronCores.

Math (per batch b):
    u  = q @ Wu            [N, R]
    vp = k @ Wv            [N, R]
    S  = u @ vp.T / sqrt(R)
    out = softmax(S) @ v   [N, D]

Shapes: B=4, N=4096, D=1024, R=32.

Sharding: data-parallel over batch x row-halves -> 8 shards. Core c handles
batch b = c // 2, rows [h*2048, (h+1)*2048) with h = c % 2. Each core gets its
q-shard and the full k/v for its batch. q/k are fed pre-transposed ([D, n]
layout) so every matmul contraction lands on the partition axis with no
on-device transposes.

Per-core device kernel (all matmuls in float32r: full PE rate, ~1e-4 rel err):
  1. uT[R, 2048]  = sum_d Wu[d, :].T qT[d, :]   (K=128 d-tiles, PSUM accum)
     vpT[R, 4096] = sum_d Wv[d, :].T kT[d, :]
  2. flash-style main loop over n-chunks of 256 rows:
       for each m-tile (128 cols): scoresT[m128, n256] = vpT_tile.T @ uT_chunk
       expT = Exp(scoresT / sqrt(R))                       (ScalarE, PSUM->SBUF)
       out_acc[n128, d512] += expT_tile.T @ v_tile         (PSUM accum over m)
       sum_acc[n128, 1]    += expT_tile.T @ ones
     out = out_acc * (1 / sum_acc)   (softmax normalization folded at the end)
"""

import numpy as np

B, N, D, R = 4, 4096, 1024, 32
NLOC = N // 2            # rows per core
RSCALE = float(1.0 / np.sqrt(np.float32(R)))

N_CHUNK = 256            # rows of scores computed per PSUM round
M_TILE = 128             # contraction tile for the AV matmul
D_HALF = 512             # PSUM bank width in fp32

LAST_RESULT = None       # test.py reads exec_time_ns etc. from here


def _build():
    from concourse import bacc, mybir
    from concourse.tile import TileContext

    f32 = mybir.dt.float32
    f32r = mybir.dt.float32r
    f16 = mybir.dt.float16
    EXP = mybir.ActivationFunctionType.Exp
    COPY = mybir.ActivationFunctionType.Copy

    nc = bacc.Bacc("TRN2", target_bir_lowering=False)

    qT = nc.dram_tensor("qT", [D, NLOC], f32r, kind="ExternalInput")
    kT = nc.dram_tensor("kT", [D, N], f32r, kind="ExternalInput")
    v = nc.dram_tensor("v", [N, D], f16, kind="ExternalInput")
    wu = nc.dram_tensor("wu", [D, R], f32r, kind="ExternalInput")
    wv = nc.dram_tensor("wv", [D, R], f32r, kind="ExternalInput")
    o = nc.dram_tensor("o", [NLOC, D], f32, kind="ExternalOutput")

    DT = D // 128         # 8 d-tiles
    NQ = NLOC // 1024     # 2 column-halves of qT
    MQ = N // 1024        # 4 column-quarters of kT
    NCH = NLOC // N_CHUNK  # 8 main-loop chunks
    MT = N // M_TILE      # 32 m tiles
    VG = 8                # v row-groups of 512
    VPG = N // VG // 128  # 4 m-tiles per v group

    with TileContext(nc) as tc:
        with tc.tile_pool(name="singles", bufs=1) as singles, \
             tc.tile_pool(name="stream", bufs=20) as stream, \
             tc.tile_pool(name="vpool", bufs=VG) as vpool, \
             tc.tile_pool(name="expp", bufs=6) as expp, \
             tc.tile_pool(name="outp", bufs=3) as outp, \
             tc.tile_pool(name="rpool", bufs=4) as rpool, \
             tc.tile_pool(name="pacc", bufs=4, space="PSUM") as pacc, \
             tc.tile_pool(name="pscore", bufs=3, space="PSUM") as pscore, \
             tc.tile_pool(name="psums", bufs=1, space="PSUM") as psums:

            # ---- constants / projection weights ----
            wu_sb = singles.tile([128, DT, R], f32r, tag="wu")
            nc.sync.dma_start(out=wu_sb, in_=wu.rearrange("(t p) r -> p t r", p=128))
            wv_sb = singles.tile([128, DT, R], f32r, tag="wv")
            nc.sync.dma_start(out=wv_sb, in_=wv.rearrange("(t p) r -> p t r", p=128))
            ones = singles.tile([128, 2], f16, tag="ones")
            nc.vector.memset(ones, 1.0)

            uT = singles.tile([R, NLOC], f32r, tag="uT")
            vpT = singles.tile([R, N], f32r, tag="vpT")

            # ---- phase 1a: uT = Wu.T @ q  (per d-tile: wu_sb[:,t,:].T @ qT_t)
            def load_qt(h):
                tiles = []
                for t in range(DT):
                    tile = stream.tile([128, 1024], f32r, tag="stream",
                                       name=f"qt{h}_{t}")
                    nc.sync.dma_start(
                        out=tile, in_=qT[t * 128:(t + 1) * 128,
                                         h * 1024:(h + 1) * 1024])
                    tiles.append(tile)
                return tiles

            qt = {}
            for t, tile in enumerate(load_qt(0)):
                qt[(t, 0)] = tile
            def u_chunk(c):
                h, off = c // 2, (c % 2) * 512
                pu = pscore.tile([R, 512], f32, tag="scores", name=f"pu{c}")
                for t in range(DT):
                    nc.tensor.matmul(pu, lhsT=wu_sb[:, t, :],
                                     rhs=qt[(t, h)][:, off:off + 512],
                                     start=(t == 0), stop=(t == DT - 1))
                nc.vector.tensor_copy(out=uT[:, c * 512:(c + 1) * 512], in_=pu)

            for c in (0, 1):
                u_chunk(c)

            # ---- v tiles, interleaved with kT quarters so neither starves
            v_sb = [None] * VG

            def load_v(g):
                vt = vpool.tile([128, VPG, D], f16, tag="v", name=f"v{g}")
                nc.sync.dma_start(
                    out=vt, in_=v[g * 512:(g + 1) * 512, :].rearrange(
                        "(t p) d -> p t d", p=128))
                v_sb[g] = vt

            load_v(0)
            load_v(1)

            # ---- phase 1b: vpT = Wv.T @ k
            for qtr in range(MQ):
                kt = []
                for t in range(DT):
                    tile = stream.tile([128, 1024], f32r, tag="stream")
                    nc.sync.dma_start(
                        out=tile, in_=kT[t * 128:(t + 1) * 128,
                                         qtr * 1024:(qtr + 1) * 1024])
                    kt.append(tile)
                if qtr < 3:
                    load_v(2 + 2 * qtr)
                    load_v(3 + 2 * qtr)
                for c2 in range(2):
                    pv = pscore.tile([R, 512], f32, tag="scores")
                    for t in range(DT):
                        nc.tensor.matmul(pv, lhsT=wv_sb[:, t, :],
                                         rhs=kt[t][:, c2 * 512:c2 * 512 + 512],
                                         start=(t == 0), stop=(t == DT - 1))
                    off = qtr * 1024 + c2 * 512
                    nc.vector.tensor_copy(out=vpT[:, off:off + 512], in_=pv)

            for t, tile in enumerate(load_qt(1)):
                qt[(t, 1)] = tile
            for c in (2, 3):
                u_chunk(c)

            # ---- phase 2: flash-style scores/softmax/AV ----
            # software-pipelined: scores/exp for m-tile mt+1 are issued before
            # the AV matmuls of m-tile mt, so ScalarE exp latency hides under
            # the previous tile's AV work on the PE.
            for ch in range(NCH):
                accs = [pacc.tile([128, D_HALF], f32, tag="acc", name=f"acc{ch}_{i}")
                        for i in range(4)]
                # both sums accumulators share one bank: start=True clears
                # has_written bank-wide, so ONLY sums[0]'s first matmul carries
                # start=True (issued before any other write to the bank); the
                # cleared has_written makes sums[1]'s first start=False matmul
                # overwrite rather than accumulate stale data
                sums_t = psums.tile([128, 4], f32, tag="sums", name=f"sum{ch}")
                sums = [sums_t[:, 0:2], sums_t[:, 2:4]]

                def scores_exp(mt):
                    ps = pscore.tile([128, N_CHUNK], f32, tag="scores",
                                     name=f"ps{ch}_{mt}")
                    nc.tensor.matmul(
                        ps, lhsT=vpT[:, mt * 128:(mt + 1) * 128],
                        rhs=uT[:, ch * N_CHUNK:(ch + 1) * N_CHUNK],
                        start=True, stop=True)
                    ex = expp.tile([128, N_CHUNK], f16, tag="ex",
                                   name=f"ex{ch}_{mt}")
                    nc.scalar.activation(out=ex, in_=ps, func=EXP, scale=RSCALE)
                    return ex

                ex_q = [scores_exp(0), scores_exp(1)]
                for mt in range(MT):
                    ex = ex_q.pop(0)
                    if mt + 2 < MT:
                        ex_q.append(scores_exp(mt + 2))
                    g, tg = mt // VPG, mt % VPG
                    first, last = (mt == 0), (mt == MT - 1)
                    for j in range(2):
                        lhs = ex[:, j * 128:(j + 1) * 128]
                        nc.tensor.matmul(accs[2 * j], lhsT=lhs,
                                         rhs=v_sb[g][:, tg, 0:D_HALF],
                                         start=first, stop=last)
                        nc.tensor.matmul(accs[2 * j + 1], lhsT=lhs,
                                         rhs=v_sb[g][:, tg, D_HALF:D],
                                         start=first, stop=last)
                        nc.tensor.matmul(sums[j], lhsT=lhs, rhs=ones,
                                         start=(first and j == 0), stop=last,
                                         skip_group_check=True)
                # normalize on DVE (keeps ScalarE free for next chunk's exp)
                for j in range(2):
                    rc = rpool.tile([128, 1], f32, tag="rc", name=f"rc{ch}_{j}")
                    nc.vector.reciprocal(rc, sums[j][:, 0:1])
                    ob = outp.tile([128, D], f32, tag="ob", name=f"ob{ch}_{j}")
                    nc.vector.tensor_scalar_mul(ob[:, 0:D_HALF], accs[2 * j], rc)
                    nc.vector.tensor_scalar_mul(ob[:, D_HALF:D], accs[2 * j + 1], rc)
                    row = ch * N_CHUNK + j * 128
                    nc.sync.dma_start(out=o[row:row + 128, :], in_=ob)

    nc.finalize()
    return nc


def kernel(q, k, v, Wu, Wv):
    global LAST_RESULT
    from concourse import bass_utils

    nc = _build()

    kTs = [np.ascontiguousarray(k[b].T) for b in range(B)]
    vs = [np.ascontiguousarray(v[b]).astype(np.float16) for b in range(B)]
    in_maps = []
    for core in range(8):
        b, h = core // 2, core % 2
        in_maps.append({
            "qT": np.ascontiguousarray(q[b].T[:, h * NLOC:(h + 1) * NLOC]),
            "kT": kTs[b],
            "v": vs[b],
            "wu": np.ascontiguousarray(Wu),
            "wv": np.ascontiguousarray(Wv),
        })

    res = bass_utils.run_bass_kernel_spmd(nc, in_maps, core_ids=list(range(8)))
    LAST_RESULT = res

    out = np.empty((B, N, D), dtype=np.float32)
    for core in range(8):
        b, h = core // 2, core % 2
        out[b, h * NLOC:(h + 1) * NLOC, :] = res.results[core]["o"]
    return out



# revision 3
# speedup vs baseline: 1.1859x; 1.1859x over previous
"""Low-rank attention kernel for Trainium2, distributed over 8 NeuronCores.

Math (per batch b):
    u  = q @ Wu            [N, R]
    vp = k @ Wv            [N, R]
    S  = u @ vp.T / sqrt(R)
    out = softmax(S) @ v   [N, D]

Shapes: B=4, N=4096, D=1024, R=32.

Sharding: data-parallel over batch x row-halves -> 8 shards. Core c handles
batch b = c // 2, rows [h*2048, (h+1)*2048) with h = c % 2. Each core gets its
q-shard and the full k/v for its batch. q/k are fed pre-transposed ([D, n]
layout, f16) so every matmul contraction lands on the partition axis with no
on-device transposes. The whole path runs in f16 (inputs are ~N(0,1); f16
keeps max rel err ~9e-4 on the final output, fp32r scores were 2 cyc/col).

Per-core device kernel:
  1. uT[R, 2048]  = sum_d Wu[d, :].T qT[d, :]   (K=128 d-tiles, PSUM accum)
     vpT[R, 4096] = sum_d Wv[d, :].T kT[d, :]
  2. main loop over n-chunks of 256 rows, key-PAIRS of 256 keys:
       scoresT pair [128, 2, 256] = two K=32 matmuls into one PSUM bank
       expT = Exp(scoresT / sqrt(R)) -> f16 [128, 2, 256]   (one ACTIVATE)
       out_acc[n128, d512] += expT_tile.T @ v_tile          (PSUM accum over m)
       sum_acc[n128, 2]    += expT_tile.T @ ones
     out = out_acc * (1 / sum_acc)   (softmax normalization folded at the end)
"""

import numpy as np

B, N, D, R = 4, 4096, 1024, 32
NLOC = N // 2            # rows per core
RSCALE = float(1.0 / np.sqrt(np.float32(R)))

N_CHUNK = 256            # rows of scores computed per PSUM round
D_HALF = 512             # PSUM bank width in fp32

LAST_RESULT = None       # test.py reads exec_time_ns etc. from here


def _build():
    from concourse import bacc, mybir
    from concourse.tile import TileContext

    f32 = mybir.dt.float32
    f16 = mybir.dt.float16
    EXP = mybir.ActivationFunctionType.Exp

    nc = bacc.Bacc("TRN2", target_bir_lowering=False)

    qT = nc.dram_tensor("qT", [D, NLOC], f16, kind="ExternalInput")
    kT = nc.dram_tensor("kT", [D, N], f16, kind="ExternalInput")
    v = nc.dram_tensor("v", [N, D], f16, kind="ExternalInput")
    wu = nc.dram_tensor("wu", [D, R], f16, kind="ExternalInput")
    wv = nc.dram_tensor("wv", [D, R], f16, kind="ExternalInput")
    o = nc.dram_tensor("o", [NLOC, D], f32, kind="ExternalOutput")

    DT = D // 128         # 8 d-tiles
    MQ = N // 1024        # 4 column-quarters of kT
    NCH = NLOC // N_CHUNK  # 8 main-loop chunks
    PAIRS = N // 256      # 16 key-pairs (256 keys each)
    VG = 8                # v row-groups of 512
    VPG = N // VG // 128  # 4 key-tiles per v group

    with TileContext(nc) as tc:
        with tc.tile_pool(name="singles", bufs=1) as singles, \
             tc.tile_pool(name="stream", bufs=16) as stream, \
             tc.tile_pool(name="vpool", bufs=VG) as vpool, \
             tc.tile_pool(name="expp", bufs=6) as expp, \
             tc.tile_pool(name="outp", bufs=3) as outp, \
             tc.tile_pool(name="rpool", bufs=4) as rpool, \
             tc.tile_pool(name="pacc", bufs=4, space="PSUM") as pacc, \
             tc.tile_pool(name="pscore", bufs=3, space="PSUM") as pscore, \
             tc.tile_pool(name="psums", bufs=1, space="PSUM") as psums:

            # ---- constants / projection weights ----
            wu_sb = singles.tile([128, DT, R], f16, tag="wu")
            nc.sync.dma_start(out=wu_sb, in_=wu.rearrange("(t p) r -> p t r", p=128))
            wv_sb = singles.tile([128, DT, R], f16, tag="wv")
            nc.sync.dma_start(out=wv_sb, in_=wv.rearrange("(t p) r -> p t r", p=128))
            ones = singles.tile([128, 2], f16, tag="ones")
            nc.vector.memset(ones, 1.0)

            uT = singles.tile([R, NLOC], f16, tag="uT")
            vpT = singles.tile([R, N], f16, tag="vpT")

            # ---- phase 1a: uT = Wu.T @ q  (per d-tile: wu_sb[:,t,:].T @ qT_t)
            def load_qt(h):
                tiles = []
                for t in range(DT):
                    tile = stream.tile([128, 1024], f16, tag="stream",
                                       name=f"qt{h}_{t}")
                    nc.sync.dma_start(
                        out=tile, in_=qT[t * 128:(t + 1) * 128,
                                         h * 1024:(h + 1) * 1024])
                    tiles.append(tile)
                return tiles

            qt = {}
            for t, tile in enumerate(load_qt(0)):
                qt[(t, 0)] = tile

            def u_chunk(c):
                h, off = c // 2, (c % 2) * 512
                pu = pscore.tile([128, 2, 256], f32, tag="scores", name=f"pu{c}")
                for t in range(DT):
                    nc.tensor.matmul(pu[0:R], lhsT=wu_sb[:, t, :],
                                     rhs=qt[(t, h)][:, off:off + 512],
                                     start=(t == 0), stop=(t == DT - 1))
                for s in range(2):
                    nc.vector.tensor_copy(
                        out=uT[:, c * 512 + s * 256:c * 512 + (s + 1) * 256],
                        in_=pu[0:R, s, :])

            for c in (0, 1):
                u_chunk(c)

            # ---- v tiles, interleaved with kT quarters so neither starves
            v_sb = [None] * VG

            def load_v(g):
                vt = vpool.tile([128, VPG, D], f16, tag="v", name=f"v{g}")
                nc.sync.dma_start(
                    out=vt, in_=v[g * 512:(g + 1) * 512, :].rearrange(
                        "(t p) d -> p t d", p=128))
                v_sb[g] = vt

            load_v(0)
            load_v(1)

            # ---- phase 1b: vpT = Wv.T @ k
            for qtr in range(MQ):
                kt = []
                for t in range(DT):
                    tile = stream.tile([128, 1024], f16, tag="stream")
                    nc.sync.dma_start(
                        out=tile, in_=kT[t * 128:(t + 1) * 128,
                                         qtr * 1024:(qtr + 1) * 1024])
                    kt.append(tile)
                if qtr < 3:
                    load_v(2 + 2 * qtr)
                    load_v(3 + 2 * qtr)
                for c2 in range(2):
                    pv = pscore.tile([128, 2, 256], f32, tag="scores")
                    for t in range(DT):
                        nc.tensor.matmul(pv[0:R], lhsT=wv_sb[:, t, :],
                                         rhs=kt[t][:, c2 * 512:c2 * 512 + 512],
                                         start=(t == 0), stop=(t == DT - 1))
                    off = qtr * 1024 + c2 * 512
                    for s in range(2):
                        nc.vector.tensor_copy(
                            out=vpT[:, off + s * 256:off + (s + 1) * 256],
                            in_=pv[0:R, s, :])

            for t, tile in enumerate(load_qt(1)):
                qt[(t, 1)] = tile
            for c in (2, 3):
                u_chunk(c)

            # ---- phase 2: flash-style scores/softmax/AV ----
            # software-pipelined: scores/exp for pair pr+2 are issued before
            # the AV matmuls of pair pr, so ScalarE exp latency hides under
            # the previous pair's AV work on the PE.
            for ch in range(NCH):
                accs = [pacc.tile([128, D_HALF], f32, tag="acc", name=f"acc{ch}_{i}")
                        for i in range(4)]
                # both sums accumulators share one bank: start=True clears
                # has_written bank-wide, so ONLY sums[0]'s first matmul carries
                # start=True; the cleared has_written makes sums[1]'s first
                # start=False matmul overwrite rather than accumulate stale data
                sums_t = psums.tile([128, 4], f32, tag="sums", name=f"sum{ch}")
                sums = [sums_t[:, 0:2], sums_t[:, 2:4]]

                def scores_exp(pr):
                    ps = pscore.tile([128, 2, N_CHUNK], f32, tag="scores",
                                     name=f"ps{ch}_{pr}")
                    for s in range(2):
                        m = 2 * pr + s
                        nc.tensor.matmul(
                            ps[:, s, :], lhsT=vpT[:, m * 128:(m + 1) * 128],
                            rhs=uT[:, ch * N_CHUNK:(ch + 1) * N_CHUNK],
                            start=True, stop=True, skip_group_check=True)
                    ex = expp.tile([128, 2, N_CHUNK], f16, tag="ex",
                                   name=f"ex{ch}_{pr}")
                    nc.scalar.activation(out=ex, in_=ps, func=EXP, scale=RSCALE)
                    return ex

                ex_q = [scores_exp(0), scores_exp(1)]
                for pr in range(PAIRS):
                    ex = ex_q.pop(0)
                    if pr + 2 < PAIRS:
                        ex_q.append(scores_exp(pr + 2))
                    g, tg = pr // 2, (pr % 2) * 2
                    for s in range(2):
                        first = (pr == 0 and s == 0)
                        last = (pr == PAIRS - 1 and s == 1)
                        for j in range(2):
                            lhs = ex[:, s, j * 128:(j + 1) * 128]
                            nc.tensor.matmul(accs[2 * j], lhsT=lhs,
                                             rhs=v_sb[g][:, tg + s, 0:D_HALF],
                                             start=first, stop=last)
                            nc.tensor.matmul(accs[2 * j + 1], lhsT=lhs,
                                             rhs=v_sb[g][:, tg + s, D_HALF:D],
                                             start=first, stop=last)
                            nc.tensor.matmul(sums[j], lhsT=lhs, rhs=ones,
                                             start=(first and j == 0), stop=last,
                                             skip_group_check=True)
                # normalize on DVE (keeps ScalarE free for next chunk's exp)
                for j in range(2):
                    rc = rpool.tile([128, 1], f32, tag="rc", name=f"rc{ch}_{j}")
                    nc.vector.reciprocal(rc, sums[j][:, 0:1])
                    ob = outp.tile([128, D], f32, tag="ob", name=f"ob{ch}_{j}")
                    nc.vector.tensor_scalar_mul(ob[:, 0:D_HALF], accs[2 * j], rc)
                    nc.vector.tensor_scalar_mul(ob[:, D_HALF:D], accs[2 * j + 1], rc)
                    row = ch * N_CHUNK + j * 128
                    nc.sync.dma_start(out=o[row:row + 128, :], in_=ob)

    nc.finalize()
    return nc


def kernel(q, k, v, Wu, Wv):
    global LAST_RESULT
    from concourse import bass_utils

    nc = _build()

    kTs = [np.ascontiguousarray(k[b].T.astype(np.float16)) for b in range(B)]
    vs = [np.ascontiguousarray(v[b]).astype(np.float16) for b in range(B)]
    wu16 = np.ascontiguousarray(Wu.astype(np.float16))
    wv16 = np.ascontiguousarray(Wv.astype(np.float16))
    in_maps = []
    for core in range(8):
        b, h = core // 2, core % 2
        in_maps.append({
            "qT": np.ascontiguousarray(
                q[b].T[:, h * NLOC:(h + 1) * NLOC].astype(np.float16)),
            "kT": kTs[b],
            "v": vs[b],
            "wu": wu16,
            "wv": wv16,
        })

    res = bass_utils.run_bass_kernel_spmd(nc, in_maps, core_ids=list(range(8)))
    LAST_RESULT = res

    out = np.empty((B, N, D), dtype=np.float32)
    for core in range(8):
        b, h = core // 2, core % 2
        out[b, h * NLOC:(h + 1) * NLOC, :] = res.results[core]["o"]
    return out


# revision 6
# speedup vs baseline: 1.2758x; 1.0757x over previous
"""Low-rank attention kernel for Trainium2, distributed over 8 NeuronCores.

Math (per batch b):
    u  = q @ Wu            [N, R]
    vp = k @ Wv            [N, R]
    S  = u @ vp.T / sqrt(R)
    out = softmax(S) @ v   [N, D]

Shapes: B=4, N=4096, D=1024, R=32.

Sharding: data-parallel over batch x row-halves -> 8 shards. Core c handles
batch b = c // 2, rows [h*2048, (h+1)*2048) with h = c % 2. Each core gets its
q-shard and the full k/v for its batch. q/k are fed pre-transposed ([D, n]
layout, f16) so every matmul contraction lands on the partition axis with no
on-device transposes. The whole path runs in f16 (inputs are ~N(0,1); f16
keeps max rel err ~9e-4 on the final output, fp32r scores were 2 cyc/col).

Per-core device kernel:
  1. uT[R, 2048]  = sum_d Wu[d, :].T qT[d, :]   (K=128 d-tiles, PSUM accum)
     vpT[R, 4096] = sum_d Wv[d, :].T kT[d, :]
  2. main loop over n-chunks of 256 rows, key-PAIRS of 256 keys:
       scoresT pair [128, 2, 256] = two K=32 matmuls into one PSUM bank
       expT = Exp(scoresT / sqrt(R)) -> f16 [128, 2, 256]   (one ACTIVATE)
       out_acc[n128, d512] += expT_tile.T @ v_tile          (PSUM accum over m)
       sum_acc[n128, 2]    += expT_tile.T @ ones
     out = out_acc * (1 / sum_acc)   (softmax normalization folded at the end)
"""

import numpy as np

B, N, D, R = 4, 4096, 1024, 32
NLOC = N // 2            # rows per core
RSCALE = float(1.0 / np.sqrt(np.float32(R)))

N_CHUNK = 256            # rows of scores computed per PSUM round
D_HALF = 512             # PSUM bank width in fp32

LAST_RESULT = None       # test.py reads exec_time_ns etc. from here


def _build():
    from concourse import bacc, mybir
    from concourse.tile import TileContext

    f32 = mybir.dt.float32
    f16 = mybir.dt.float16
    EXP = mybir.ActivationFunctionType.Exp

    nc = bacc.Bacc("TRN2", target_bir_lowering=False)

    qT = nc.dram_tensor("qT", [D, NLOC], f16, kind="ExternalInput")
    kT = nc.dram_tensor("kT", [D, N], f16, kind="ExternalInput")
    v = nc.dram_tensor("v", [N, D], f16, kind="ExternalInput")
    wu = nc.dram_tensor("wu", [D, R], f16, kind="ExternalInput")
    wv = nc.dram_tensor("wv", [D, R], f16, kind="ExternalInput")
    o = nc.dram_tensor("o", [NLOC, D], f32, kind="ExternalOutput")

    DT = D // 128         # 8 d-tiles
    MQ = N // 1024        # 4 column-quarters of kT
    NCH = NLOC // N_CHUNK  # 8 main-loop chunks
    PAIRS = N // 256      # 16 key-pairs (256 keys each)
    VG = 8                # v row-groups of 512
    VPG = N // VG // 128  # 4 key-tiles per v group

    with TileContext(nc) as tc:
        with tc.tile_pool(name="singles", bufs=1) as singles, \
             tc.tile_pool(name="stream", bufs=32) as stream, \
             tc.tile_pool(name="vpool", bufs=VG) as vpool, \
             tc.tile_pool(name="expp", bufs=6) as expp, \
             tc.tile_pool(name="outp", bufs=2) as outp, \
             tc.tile_pool(name="rpool", bufs=4) as rpool, \
             tc.tile_pool(name="pacc", bufs=4, space="PSUM") as pacc, \
             tc.tile_pool(name="pscore", bufs=3, space="PSUM") as pscore, \
             tc.tile_pool(name="psums", bufs=1, space="PSUM") as psums:

            # ---- constants / projection weights ----
            wu_sb = singles.tile([128, DT, R], f16, tag="wu")
            nc.sync.dma_start(out=wu_sb, in_=wu.rearrange("(t p) r -> p t r", p=128))
            wv_sb = singles.tile([128, DT, R], f16, tag="wv")
            nc.sync.dma_start(out=wv_sb, in_=wv.rearrange("(t p) r -> p t r", p=128))
            ones = singles.tile([128, 2], f16, tag="ones")
            nc.vector.memset(ones, 1.0)

            uT = singles.tile([R, NLOC], f16, tag="uT")
            vpT = singles.tile([R, N], f16, tag="vpT")

            # ---- all input DMAs issued up front, in critical-path order:
            # kT q0 gates the first projection; qT h0 gates uT chunk 0; the
            # rest streams in while the PE works through projections + chunk 0.
            kt = {}      # (qtr, t) -> tile
            qt = {}      # (t, h) -> tile
            v_sb = [None] * VG

            def load_kq(qtr):
                for t in range(DT):
                    tile = stream.tile([128, 1024], f16, tag="stream",
                                       name=f"kt{qtr}_{t}")
                    nc.sync.dma_start(
                        out=tile, in_=kT[t * 128:(t + 1) * 128,
                                         qtr * 1024:(qtr + 1) * 1024])
                    kt[(qtr, t)] = tile

            def load_qh(h):
                for t in range(DT):
                    tile = stream.tile([128, 1024], f16, tag="stream",
                                       name=f"qt{h}_{t}")
                    nc.sync.dma_start(
                        out=tile, in_=qT[t * 128:(t + 1) * 128,
                                         h * 1024:(h + 1) * 1024])
                    qt[(t, h)] = tile

            def load_v(g):
                vt = vpool.tile([128, VPG, D], f16, tag="v", name=f"v{g}")
                nc.sync.dma_start(
                    out=vt, in_=v[g * 512:(g + 1) * 512, :].rearrange(
                        "(t p) d -> p t d", p=128))
                v_sb[g] = vt

            load_kq(0)
            load_qh(0)
            load_kq(1)
            load_v(0)
            load_v(1)
            load_kq(2)
            load_v(2)
            load_v(3)
            load_kq(3)
            load_v(4)
            load_v(5)
            load_v(6)
            load_v(7)
            load_qh(1)

            # ---- projection helpers ----
            def u_chunk(c):
                h, off = c // 2, (c % 2) * 512
                pu = pscore.tile([128, 2, 256], f32, tag="scores", name=f"pu{c}")
                for t in range(DT):
                    nc.tensor.matmul(pu[0:R], lhsT=wu_sb[:, t, :],
                                     rhs=qt[(t, h)][:, off:off + 512],
                                     start=(t == 0), stop=(t == DT - 1))
                for s in range(2):
                    nc.vector.tensor_copy(
                        out=uT[:, c * 512 + s * 256:c * 512 + (s + 1) * 256],
                        in_=pu[0:R, s, :])

            def vp_quarter(qtr):
                for c2 in range(2):
                    pv = pscore.tile([128, 2, 256], f32, tag="scores")
                    for t in range(DT):
                        nc.tensor.matmul(pv[0:R], lhsT=wv_sb[:, t, :],
                                         rhs=kt[(qtr, t)][:, c2 * 512:c2 * 512 + 512],
                                         start=(t == 0), stop=(t == DT - 1))
                    off = qtr * 1024 + c2 * 512
                    for s in range(2):
                        nc.vector.tensor_copy(
                            out=vpT[:, off + s * 256:off + (s + 1) * 256],
                            in_=pv[0:R, s, :])

            # ---- phase 2: flash-style scores/softmax/AV ----
            # software-pipelined: scores/exp for pair pr+2 are issued before
            # the AV matmuls of pair pr, so ScalarE exp latency hides under
            # the previous pair's AV work on the PE. hooks[pr] lets chunk 0
            # interleave the remaining projection quarters at the right spots.
            def emit_chunk(ch, hooks=None):
                accs = [pacc.tile([128, D_HALF], f32, tag="acc", name=f"acc{ch}_{i}")
                        for i in range(4)]
                # both sums accumulators share one bank: start=True clears
                # has_written bank-wide, so ONLY sums[0]'s first matmul carries
                # start=True; the cleared has_written makes sums[1]'s first
                # start=False matmul overwrite rather than accumulate stale data
                sums_t = psums.tile([128, 4], f32, tag="sums", name=f"sum{ch}")
                sums = [sums_t[:, 0:2], sums_t[:, 2:4]]

                def scores_exp(pr):
                    ps = pscore.tile([128, 2, N_CHUNK], f32, tag="scores",
                                     name=f"ps{ch}_{pr}")
                    for s in range(2):
                        m = 2 * pr + s
                        nc.tensor.matmul(
                            ps[:, s, :], lhsT=vpT[:, m * 128:(m + 1) * 128],
                            rhs=uT[:, ch * N_CHUNK:(ch + 1) * N_CHUNK],
                            start=True, stop=True, skip_group_check=True)
                    ex = expp.tile([128, 2, N_CHUNK], f16, tag="ex",
                                   name=f"ex{ch}_{pr}")
                    nc.scalar.activation(out=ex, in_=ps, func=EXP, scale=RSCALE)
                    return ex

                ex_q = [scores_exp(0), scores_exp(1)]
                for pr in range(PAIRS):
                    if hooks and pr in hooks:
                        hooks[pr]()
                    ex = ex_q.pop(0)
                    if pr + 2 < PAIRS:
                        ex_q.append(scores_exp(pr + 2))
                    g, tg = pr // 2, (pr % 2) * 2
                    for s in range(2):
                        first = (pr == 0 and s == 0)
                        last = (pr == PAIRS - 1 and s == 1)
                        for j in range(2):
                            lhs = ex[:, s, j * 128:(j + 1) * 128]
                            nc.tensor.matmul(accs[2 * j], lhsT=lhs,
                                             rhs=v_sb[g][:, tg + s, 0:D_HALF],
                                             start=first, stop=last,
                                             skip_group_check=True)
                            nc.tensor.matmul(accs[2 * j + 1], lhsT=lhs,
                                             rhs=v_sb[g][:, tg + s, D_HALF:D],
                                             start=first, stop=last,
                                             skip_group_check=True)
                            nc.tensor.matmul(sums[j], lhsT=lhs, rhs=ones,
                                             start=(first and j == 0), stop=last,
                                             skip_group_check=True)
                # normalize on DVE (keeps ScalarE free for next chunk's exp)
                ob = outp.tile([128, 2, D], f32, tag="ob", name=f"ob{ch}")
                for j in range(2):
                    rc = rpool.tile([128, 1], f32, tag="rc", name=f"rc{ch}_{j}")
                    nc.vector.reciprocal(rc, sums[j][:, 0:1])
                    nc.vector.tensor_scalar_mul(ob[:, j, 0:D_HALF], accs[2 * j], rc)
                    nc.vector.tensor_scalar_mul(ob[:, j, D_HALF:D], accs[2 * j + 1], rc)
                nc.sync.dma_start(
                    out=o[ch * N_CHUNK:(ch + 1) * N_CHUNK, :].rearrange(
                        "(j p) d -> p j d", p=128),
                    in_=ob)

            # PE issue order: vp q0 first (kT q0 is the first DMA to land),
            # then u chunk 0/1, vp q1, then chunk 0 with vp q2/q3 and the
            # remaining u chunks interleaved where their data has arrived.
            vp_quarter(0)
            for c in (0, 1):
                u_chunk(c)
            vp_quarter(1)
            emit_chunk(0, hooks={
                4: lambda: vp_quarter(2),
                8: lambda: vp_quarter(3),
                12: lambda: (u_chunk(2), u_chunk(3)),
            })
            for ch in range(1, NCH):
                emit_chunk(ch)

    nc.finalize()
    return nc


def kernel(q, k, v, Wu, Wv):
    global LAST_RESULT
    from concourse import bass_utils

    nc = _build()

    kTs = [np.ascontiguousarray(k[b].T.astype(np.float16)) for b in range(B)]
    vs = [np.ascontiguousarray(v[b]).astype(np.float16) for b in range(B)]
    wu16 = np.ascontiguousarray(Wu.astype(np.float16))
    wv16 = np.ascontiguousarray(Wv.astype(np.float16))
    in_maps = []
    for core in range(8):
        b, h = core // 2, core % 2
        in_maps.append({
            "qT": np.ascontiguousarray(
                q[b].T[:, h * NLOC:(h + 1) * NLOC].astype(np.float16)),
            "kT": kTs[b],
            "v": vs[b],
            "wu": wu16,
            "wv": wv16,
        })

    res = bass_utils.run_bass_kernel_spmd(nc, in_maps, core_ids=list(range(8)))
    LAST_RESULT = res

    out = np.empty((B, N, D), dtype=np.float32)
    for core in range(8):
        b, h = core // 2, core % 2
        out[b, h * NLOC:(h + 1) * NLOC, :] = res.results[core]["o"]
    return out
